# revision 1
# baseline (speedup 1.0000x reference)
"""MLA + DeepSeekMoE block on 8 trn2 NeuronCores (Bass/Tile SPMD).

Token-sharded across 8 cores (512 tokens each, causally balanced stripes).
Attention uses the MLA absorption trick: scores/outputs computed against the
shared 320-dim KV latent, so the only collective is one AllGather of the
normalized latent per batch group of 4 cores.
"""

import numpy as np
import ml_dtypes

import concourse.bacc as bacc
import concourse.mybir as mybir
from concourse.tile import TileContext
from concourse.bass_utils import run_bass_kernel_spmd

# ---- problem constants ----
D = 1024; H = 8; QLR = 384; KVLR = 256; NOPE = 128; ROPE = 64; VD = 128
E = 8; TOPK = 2; INTER = 512; NSH = 2; B = 2; T = 2048; QKD = NOPE + ROPE
N = B * T
NCORES = 8
NLOC = N // NCORES          # 512 tokens per core
P = 128
EPS = 1e-6
SCALE = 1.0 / np.sqrt(QKD)
NEG = -1e9
DEBUG = False

F32 = mybir.dt.float32
F32R = mybir.dt.float32r
BF16 = mybir.dt.bfloat16


def _r(ap):
    return ap  # operand dtypes are declared fp32r where needed


def _kmap(a):
    """abs k-tile (0..15, 128 tokens each) -> (group-local core block 0..3,
    local tile 0..3 within that core's 512-token contribution)."""
    s = a // 2
    blk = s if s < 4 else 7 - s
    ktl = (a % 2) + (0 if s < 4 else 2)
    return blk, ktl


def _rope_perm():
    return np.concatenate([np.arange(0, ROPE, 2), np.arange(1, ROPE, 2)])


def _core_positions(c):
    """batch-local positions (512,) of core c's tokens."""
    j = c % 4
    return np.concatenate([np.arange(j * 256, (j + 1) * 256),
                           np.arange((7 - j) * 256, (8 - j) * 256)])


def _tile_w(w):
    """[K, F] row-major -> [128, K//128, F] partition-major contiguous."""
    K, F = w.shape
    return np.ascontiguousarray(w.reshape(K // P, P, F).transpose(1, 0, 2))


# ============================ device program ============================

def build():
    from contextlib import ExitStack
    nc = bacc.Bacc(name="mla_moe")

    # ---- I/O ----
    xT = nc.dram_tensor("xT", [D, NLOC], F32R, kind="ExternalInput")
    cosT = nc.dram_tensor("cosT", [ROPE // 2, NLOC], F32, kind="ExternalInput")
    sinT = nc.dram_tensor("sinT", [ROPE // 2, NLOC], F32, kind="ExternalInput")
    mask1 = nc.dram_tensor("mask1", [8, P, NLOC], BF16, kind="ExternalInput")
    mask2 = nc.dram_tensor("mask2", [8, P, 256], BF16, kind="ExternalInput")
    ident = nc.dram_tensor("ident", [P, P], F32, kind="ExternalInput")
    ones32 = nc.dram_tensor("ones32", [P, 1], F32, kind="ExternalInput")
    onesbf = nc.dram_tensor("onesbf", [P, 1], BF16, kind="ExternalInput")
    lat_w = nc.dram_tensor("lat_w", [P, 8, QLR + KVLR + ROPE], F32R, kind="ExternalInput")
    q_up = nc.dram_tensor("q_up", [P, 3, H * QKD], F32R, kind="ExternalInput")
    kv_up = nc.dram_tensor("kv_up", [P, 2, H * (NOPE + VD)], F32R, kind="ExternalInput")
    wv_w = nc.dram_tensor("wv_w", [P, 2, H * VD], F32R, kind="ExternalInput")
    cproj_w = nc.dram_tensor("cproj_w", [P, 8, D], F32R, kind="ExternalInput")
    gate_w = nc.dram_tensor("gate_w", [P, 8, E], F32, kind="ExternalInput")
    shw1 = nc.dram_tensor("shw1", [P, 8, INTER * NSH], F32R, kind="ExternalInput")
    shw3 = nc.dram_tensor("shw3", [P, 8, INTER * NSH], F32R, kind="ExternalInput")
    shw2 = nc.dram_tensor("shw2", [P, 8, D], F32R, kind="ExternalInput")
    ew1 = nc.dram_tensor("ew1", [E, P, 8, INTER], F32R, kind="ExternalInput")
    ew3 = nc.dram_tensor("ew3", [E, P, 8, INTER], F32R, kind="ExternalInput")
    ew2 = nc.dram_tensor("ew2", [E, P, 4, D], F32R, kind="ExternalInput")
    sel8 = nc.dram_tensor("sel8", [E, E * P], F32, kind="ExternalInput")
    out_xT = nc.dram_tensor("out_xT", [D, NLOC], F32, kind="ExternalOutput")

    cc_in = nc.dram_tensor("cc_in", [384, NLOC], F32R)
    cc_out = nc.dram_tensor("cc_out", [4 * 384, NLOC], F32R)
    RG = [[0, 1, 2, 3], [4, 5, 6, 7]]

    AL = mybir.AluOpType
    AF = mybir.ActivationFunctionType

    with TileContext(nc) as tc, \
         tc.tile_pool(name="const", bufs=1) as p_const, \
         tc.tile_pool(name="rows", bufs=1) as p_rows, \
         tc.tile_pool(name="psc", bufs=2) as p_sc:

        # ---- long-lived pool groups, closed manually at phase boundaries ----
        # right-side stack LIFO: opens x, y, q, kv, mask;
        # closes mask, kv, q (post-attention), y, x (post-cproj)
        g_x = ExitStack();    p_x = g_x.enter_context(tc.tile_pool(name="px", bufs=1, side="right"))
        g_y = ExitStack();    p_y = g_y.enter_context(tc.tile_pool(name="py", bufs=1, side="right"))
        g_q = ExitStack()
        g_kv = ExitStack()
        g_mask = ExitStack()

        identf = p_const.tile([P, P], F32, tag="identf")
        nc.sync.dma_start(out=identf[:], in_=ident[:])
        ident_sb = p_const.tile([P, P], F32R, tag="identr")
        nc.vector.tensor_copy(out=ident_sb[:], in_=identf[:])
        ones32r = p_const.tile([P, 1], F32R, tag="ones32r")
        # (filled from ones32_sb below)
        ones32_sb = p_const.tile([P, 1], F32, tag="ones32")
        nc.sync.dma_start(out=ones32_sb[:], in_=ones32[:])
        nc.vector.tensor_copy(out=ones32r[:], in_=ones32_sb[:])
        onesbf_sb = p_const.tile([P, 1], BF16, tag="onesbf")
        nc.sync.dma_start(out=onesbf_sb[:], in_=onesbf[:])
        cos_sb = p_const.tile([ROPE // 2, NLOC], F32, tag="cos")
        nc.sync.dma_start(out=cos_sb[:], in_=cosT[:])
        sin_sb = p_const.tile([ROPE // 2, NLOC], F32, tag="sin")
        nc.sync.dma_start(out=sin_sb[:], in_=sinT[:])
        eps1 = p_const.tile([1, 1], F32, tag="eps1")
        nc.vector.memset(eps1[:], EPS)
        xT_sb = p_x.tile([P, 8, NLOC], F32R, tag="xT")
        nc.sync.dma_start(out=xT_sb[:], in_=xT.rearrange("(a p) n -> p a n", p=P))

        rows_sb = p_rows.tile([1, 3, NLOC], F32, tag="rows")
        # row slots: 0=current norm scale, 1=tmp, 2=attn recip

        ones1_sb = p_const.tile([1, P], F32, tag="ones1")
        nc.vector.memset(ones1_sb[:], 1.0)
        sel8_sb = p_const.tile([E, E * P], F32, tag="sel8")
        nc.sync.dma_start(out=sel8_sb[:], in_=sel8[:])

        def brow(ps_bc, sb_pool, row_ap, n=NLOC, tag="bc"):
            # broadcast a [1, n] row to [128, n]: PE ones-matmul -> psum -> sbuf
            bc = ps_bc.tile([P, NLOC], F32, tag=tag)
            nc.tensor.matmul(bc[:, :n], ones1_sb[:], row_ap, start=True, stop=True)
            sb = sb_pool.tile([P, NLOC], F32, tag="bcsb")
            nc.vector.tensor_copy(out=sb[:, :n], in_=bc[:, :n])
            return sb

        # ================= phase A: latents, norms, rope, gather, q =================
        with tc.tile_pool(name="weq", bufs=1) as p_weq, \
             tc.tile_pool(name="acta", bufs=1) as p_actA, \
             tc.tile_pool(name="qsc", bufs=2) as p_qsc:

            qup_sb = p_weq.tile([P, 3, H * QKD], F32R, tag="qup")
            nc.sync.dma_start(out=qup_sb[:], in_=q_up[:])
            kvup_sb = p_weq.tile([P, 2, H * (NOPE + VD)], F32R, tag="kvup")
            nc.sync.dma_start(out=kvup_sb[:], in_=kv_up[:])
            qln = p_actA.tile([P, 3, NLOC], F32R, tag="qln")
            kvn = p_actA.tile([P, 2, NLOC], F32R, tag="kvn")
            kr = p_actA.tile([ROPE, NLOC], F32R, tag="kr")
            scr = p_actA.tile([ROPE // 2, NLOC], F32, tag="krs")

            with tc.tile_pool(name="welat", bufs=1) as p_welat, \
                 tc.tile_pool(name="plat", bufs=1) as p_lat, \
                 tc.tile_pool(name="pslat", bufs=6, space="PSUM") as ps_lat, \
                 tc.tile_pool(name="psrow", bufs=1, space="PSUM") as ps_row, \
                 tc.tile_pool(name="psbc", bufs=1, space="PSUM") as ps_bc:

                latw_sb = p_welat.tile([P, 8, QLR + KVLR + ROPE], F32R, tag="latw")
                nc.sync.dma_start(out=latw_sb[:], in_=lat_w[:])

                # rms1 sumsq via bf16 squares + ones matmul
                ss_ps = ps_row.tile([1, NLOC], F32, tag="ss")
                for ds in range(8):
                    xsq = p_sc.tile([P, NLOC], BF16, tag="xsq")
                    nc.vector.tensor_mul(out=xsq[:], in0=xT_sb[:, ds, :], in1=xT_sb[:, ds, :])
                    nc.tensor.matmul(ss_ps[:], onesbf_sb[:], xsq[:],
                                     start=(ds == 0), stop=(ds == 7))
                nc.scalar.activation(out=rows_sb[:, 1, :], in_=ss_ps[:],
                                     func=AF.Sqrt, bias=eps1[:], scale=1.0 / D)
                nc.vector.reciprocal(out=rows_sb[:, 0, :], in_=rows_sb[:, 1, :])

                # latent matmul (scale by s1 on copyback)
                latT = p_lat.tile([P, 6, NLOC], F32, tag="latT")
                s1b = brow(ps_bc, p_sc, rows_sb[:, 0, :])
                fts = [(0, 128), (128, 128), (256, 128), (384, 128), (512, 128), (640, 64)]
                for ft, (f0, fsz) in enumerate(fts):
                    lp = ps_lat.tile([P, NLOC], F32, tag="lat")
                    for ds in range(8):
                        nc.tensor.matmul(lp[:fsz], _r(latw_sb[:, ds, f0:f0 + fsz]),
                                         _r(xT_sb[:, ds, :]),
                                         start=(ds == 0), stop=(ds == 7))
                    nc.vector.tensor_tensor(out=latT[:fsz, ft, :], in0=lp[:fsz],
                                            in1=s1b[:fsz], op=AL.mult)

                # q-norm scale (fold attention score scale in)
                sq_ps = ps_row.tile([1, NLOC], F32, tag="ss")
                for t in range(3):
                    xsq = p_sc.tile([P, NLOC], BF16, tag="xsq")
                    nc.vector.tensor_mul(out=xsq[:], in0=latT[:, t, :], in1=latT[:, t, :])
                    nc.tensor.matmul(sq_ps[:], onesbf_sb[:], xsq[:],
                                     start=(t == 0), stop=(t == 2))
                nc.scalar.activation(out=rows_sb[:, 1, :], in_=sq_ps[:],
                                     func=AF.Sqrt, bias=eps1[:], scale=1.0 / QLR)
                nc.vector.reciprocal(out=rows_sb[:, 0, :], in_=rows_sb[:, 1, :])
                nc.vector.tensor_scalar_mul(out=rows_sb[:, 0, :], in0=rows_sb[:, 0, :],
                                            scalar1=float(SCALE))
                sqb = brow(ps_bc, p_sc, rows_sb[:, 0, :])
                for t in range(3):
                    nc.vector.tensor_tensor(out=qln[:, t, :], in0=latT[:, t, :],
                                            in1=sqb[:], op=AL.mult)

                # kv-norm scale
                skv_ps = ps_row.tile([1, NLOC], F32, tag="ss")
                for i, t in enumerate((3, 4)):
                    xsq = p_sc.tile([P, NLOC], BF16, tag="xsq")
                    nc.vector.tensor_mul(out=xsq[:], in0=latT[:, t, :], in1=latT[:, t, :])
                    nc.tensor.matmul(skv_ps[:], onesbf_sb[:], xsq[:],
                                     start=(i == 0), stop=(i == 1))
                nc.scalar.activation(out=rows_sb[:, 1, :], in_=skv_ps[:],
                                     func=AF.Sqrt, bias=eps1[:], scale=1.0 / KVLR)
                nc.vector.reciprocal(out=rows_sb[:, 0, :], in_=rows_sb[:, 1, :])
                skvb = brow(ps_bc, p_sc, rows_sb[:, 0, :])
                for t in range(2):
                    nc.vector.tensor_tensor(out=kvn[:, t, :], in0=latT[:, 3 + t, :],
                                            in1=skvb[:], op=AL.mult)

                # k_r rope (rows deinterleaved by host weight permutation).
                # DVE needs equal SBUF base partitions: stage odd half at base 0.
                ev = latT[0:32, 5, :]
                odc = p_actA.tile([ROPE // 2, NLOC], F32, tag="odc")
                scr2 = p_actA.tile([ROPE // 2, NLOC], F32, tag="krs2")
                nc.vector.tensor_copy(out=odc[:], in_=latT[32:64, 5, :])
                nc.vector.tensor_mul(out=kr[0:32], in0=ev, in1=cos_sb[:])
                nc.vector.tensor_mul(out=scr[:], in0=odc[:], in1=sin_sb[:])
                nc.vector.tensor_sub(out=kr[0:32], in0=kr[0:32], in1=scr[:])
                nc.vector.tensor_mul(out=scr[:], in0=ev, in1=sin_sb[:])
                nc.vector.tensor_mul(out=scr2[:], in0=odc[:], in1=cos_sb[:])
                nc.vector.tensor_add(out=scr[:], in0=scr[:], in1=scr2[:])
                nc.vector.tensor_copy(out=kr[32:64], in_=scr[:])

            # contribution -> internal DRAM -> AllGather (per batch group)
            nc.sync.dma_start(out=cc_in[0:128], in_=kvn[:, 0, :])
            nc.sync.dma_start(out=cc_in[128:256], in_=kvn[:, 1, :])
            nc.sync.dma_start(out=cc_in[256:320], in_=kr[:])
            nc.sync.dma_start(out=cc_in[320:384], in_=kr[:])  # pad, never read
            nc.gpsimd.collective_compute(
                "AllGather", AL.bypass, ins=[cc_in[:]], outs=[cc_out[:]],
                replica_groups=RG)

            # q side per head (overlaps the AllGather)
            p_q = g_q.enter_context(tc.tile_pool(name="pq", bufs=1, side="right"))
            qabs = p_q.tile([P, 2 * H, NLOC], F32R, tag="qabs")
            qrope = p_q.tile([ROPE, H, NLOC], F32R, tag="qrope")
            with tc.tile_pool(name="pstp", bufs=4, space="PSUM") as ps_tp, \
                 tc.tile_pool(name="psqp", bufs=3, space="PSUM") as ps_qp:
                for h in range(H):
                    qn_ps = ps_qp.tile([P, NLOC], F32, tag="qp")
                    for t in range(3):
                        nc.tensor.matmul(qn_ps[:], _r(qup_sb[:, t, h * QKD:h * QKD + NOPE]),
                                         _r(qln[:, t, :]), start=(t == 0), stop=(t == 2))
                    qn_sb = p_qsc.tile([P, NLOC], F32R, tag="qnsb")
                    nc.vector.tensor_copy(out=qn_sb[:], in_=qn_ps[:])
                    wkT = p_qsc.tile([P, KVLR], F32R, tag="wkT")
                    for i in range(2):
                        tp = ps_tp.tile([P, P], F32R, tag="tp")
                        nc.tensor.transpose(tp[:], kvup_sb[:, i, h * (NOPE + VD):h * (NOPE + VD) + NOPE],
                                            ident_sb[:])
                        nc.vector.tensor_copy(out=wkT[:, i * 128:(i + 1) * 128], in_=tp[:])
                    for i in range(2):
                        qa_ps = ps_qp.tile([P, NLOC], F32, tag="qp")
                        nc.tensor.matmul(qa_ps[:], _r(wkT[:, i * 128:(i + 1) * 128]),
                                         _r(qn_sb[:]), start=True, stop=True)
                        nc.vector.tensor_copy(out=qabs[:, h * 2 + i, :], in_=qa_ps[:])
                    qr_ps = ps_qp.tile([P, NLOC], F32, tag="qp")
                    for t in range(3):
                        nc.tensor.matmul(qr_ps[:ROPE],
                                         _r(qup_sb[:, t, h * QKD + NOPE:(h + 1) * QKD]),
                                         _r(qln[:, t, :]), start=(t == 0), stop=(t == 2))
                    qsc = p_qsc.tile([ROPE // 2, NLOC], F32, tag="qsc")
                    qsc2 = p_qsc.tile([ROPE // 2, NLOC], F32, tag="qsc2")
                    nc.vector.tensor_mul(out=qrope[0:32, h, :], in0=qr_ps[0:32], in1=cos_sb[:])
                    nc.vector.tensor_mul(out=qsc[:], in0=qr_ps[32:64], in1=sin_sb[:])
                    nc.vector.tensor_sub(out=qrope[0:32, h, :], in0=qrope[0:32, h, :], in1=qsc[:])
                    nc.vector.tensor_mul(out=qsc[:], in0=qr_ps[0:32], in1=sin_sb[:])
                    nc.vector.tensor_mul(out=qsc2[:], in0=qr_ps[32:64], in1=cos_sb[:])
                    nc.vector.tensor_add(out=qsc[:], in0=qsc[:], in1=qsc2[:])
                    nc.vector.tensor_copy(out=qrope[32:64, h, :], in_=qsc[:])

        # ================= gathered KV: fm blocks + tm transposes =================
        p_kv = g_kv.enter_context(tc.tile_pool(name="pkv", bufs=1, side="right"))
        kvfm = p_kv.tile([P, 12, NLOC], F32R, tag="kvfm")
        nc.sync.dma_start(out=kvfm[:], in_=cc_out.rearrange("(a p) n -> p a n", p=P))
        kvtm = p_kv.tile([P, 16, KVLR], F32R, tag="kvtm")
        with tc.tile_pool(name="pstp", bufs=4, space="PSUM") as ps_tp:
            for blk in range(4):
                for dsi in range(2):
                    for q4 in range(4):
                        tp = ps_tp.tile([P, P], F32R, tag="tp")
                        nc.tensor.transpose(tp[:], kvfm[:, 3 * blk + dsi, q4 * 128:(q4 + 1) * 128],
                                            ident_sb[:])
                        nc.vector.tensor_copy(
                            out=kvtm[:, blk * 4 + q4, dsi * 128:(dsi + 1) * 128], in_=tp[:])

        # ================= attention =================
        p_mask = g_mask.enter_context(tc.tile_pool(name="pmask", bufs=1, side="right"))
        m1_sb = p_mask.tile([P, 8, NLOC], BF16, tag="m1")
        nc.sync.dma_start(out=m1_sb[:], in_=mask1.rearrange("a p n -> p a n"))
        m2_sb = p_mask.tile([P, 8, 256], BF16, tag="m2")
        nc.sync.dma_start(out=m2_sb[:], in_=mask2.rearrange("a p n -> p a n"))
        wv_sb = p_y.tile([P, 2, H * VD], F32R, tag="wv")
        nc.sync.dma_start(out=wv_sb[:], in_=wv_w[:])
        yT = p_y.tile([P, 8, NLOC], F32R, tag="yT")

        with tc.tile_pool(name="psst", bufs=2, space="PSUM") as ps_st, \
             tc.tile_pool(name="psol", bufs=1, space="PSUM") as ps_ol, \
             tc.tile_pool(name="psden", bufs=1, space="PSUM") as ps_den, \
             tc.tile_pool(name="patt", bufs=2) as p_att:
            for h in range(H):
                olA0 = ps_ol.tile([P, 256], F32, tag="olA0")
                olA1 = ps_ol.tile([P, 256], F32, tag="olA1")
                olB0 = ps_ol.tile([P, 256], F32, tag="olB0")
                olB1 = ps_ol.tile([P, 256], F32, tag="olB1")
                denA = ps_den.tile([1, 256], F32, tag="denA")
                denB = ps_den.tile([1, 256], F32, tag="denB")
                for kt in range(16):
                    blk, ktl = _kmap(kt)
                    slab1 = kt < 8
                    w = NLOC if slab1 else 256
                    qof = 0 if slab1 else 256
                    kc = slice(ktl * 128, (ktl + 1) * 128)
                    st = ps_st.tile([P, NLOC], F32, tag="st")
                    nc.tensor.matmul(st[:, :w], _r(kvfm[:, 3 * blk + 0, kc]),
                                     _r(qabs[:, 2 * h + 0, qof:NLOC]), start=True, stop=False)
                    nc.tensor.matmul(st[:, :w], _r(kvfm[:, 3 * blk + 1, kc]),
                                     _r(qabs[:, 2 * h + 1, qof:NLOC]), start=False, stop=False)
                    nc.tensor.matmul(st[:, :w], _r(kvfm[0:ROPE, 3 * blk + 2, kc]),
                                     _r(qrope[:, h, qof:NLOC]), start=False, stop=True)
                    msb = m1_sb[:, kt, :] if slab1 else m2_sb[:, kt - 8, :]
                    nc.vector.tensor_tensor(out=st[:, :w], in0=st[:, :w], in1=msb, op=AL.add)
                    pt = p_att.tile([P, NLOC], F32R, tag="P")
                    nc.scalar.activation(out=pt[:, :w], in_=st[:, :w], func=AF.Exp)
                    tmi = blk * 4 + ktl
                    if slab1:
                        nc.tensor.matmul(denA[:], ones32r[:], _r(pt[:, 0:256]),
                                         start=(kt == 0), stop=(kt == 7))
                        nc.tensor.matmul(olA0[:], _r(kvtm[:, tmi, 0:128]), _r(pt[:, 0:256]),
                                         start=(kt == 0), stop=(kt == 7))
                        nc.tensor.matmul(olA1[:], _r(kvtm[:, tmi, 128:256]), _r(pt[:, 0:256]),
                                         start=(kt == 0), stop=(kt == 7))
                        pB = pt[:, 256:NLOC]
                    else:
                        pB = pt[:, 0:256]
                    nc.tensor.matmul(denB[:], ones32r[:], _r(pB),
                                     start=(kt == 0), stop=(kt == 15))
                    nc.tensor.matmul(olB0[:], _r(kvtm[:, tmi, 0:128]), _r(pB),
                                     start=(kt == 0), stop=(kt == 15))
                    nc.tensor.matmul(olB1[:], _r(kvtm[:, tmi, 128:256]), _r(pB),
                                     start=(kt == 0), stop=(kt == 15))
                nc.vector.reciprocal(out=rows_sb[:, 2, 0:256], in_=denA[:])
                nc.vector.reciprocal(out=rows_sb[:, 2, 256:NLOC], in_=denB[:])
                olA_sb = p_att.tile([P, NLOC], F32R, tag="olAsb")
                olB_sb = p_att.tile([P, NLOC], F32R, tag="olBsb")
                rA = brow(ps_st, p_att, rows_sb[:, 2, 0:256], 256, tag="st")
                rB = brow(ps_st, p_att, rows_sb[:, 2, 256:NLOC], 256, tag="st")
                for half, ol in ((0, olA0), (1, olA1)):
                    nc.vector.tensor_tensor(out=olA_sb[:, half * 256:(half + 1) * 256],
                                            in0=ol[:], in1=rA[:, 0:256], op=AL.mult)
                for half, ol in ((0, olB0), (1, olB1)):
                    nc.vector.tensor_tensor(out=olB_sb[:, half * 256:(half + 1) * 256],
                                            in0=ol[:], in1=rB[:, 0:256], op=AL.mult)
                for half, src in ((0, olA_sb), (1, olB_sb)):
                    yp = ps_st.tile([P, NLOC], F32, tag="st")
                    nc.tensor.matmul(yp[:, 0:256], _r(wv_sb[:, 0, h * VD:(h + 1) * VD]),
                                     _r(src[:, 0:256]), start=True, stop=False)
                    nc.tensor.matmul(yp[:, 0:256], _r(wv_sb[:, 1, h * VD:(h + 1) * VD]),
                                     _r(src[:, 256:NLOC]), start=False, stop=True)
                    nc.vector.tensor_copy(out=yT[:, h, half * 256:(half + 1) * 256],
                                          in_=yp[:, 0:256])

        g_mask.close()
        g_kv.close()
        g_q.close()

        # ================= c_proj + residual, rms2, gate, MoE =================
        with tc.tile_pool(name="pwmlp", bufs=3) as p_wmlp, \
             tc.tile_pool(name="px2", bufs=1) as p_x2, \
             tc.tile_pool(name="psmm", bufs=4, space="PSUM") as ps_mm, \
             tc.tile_pool(name="psrow2", bufs=1, space="PSUM") as ps_row2, \
             tc.tile_pool(name="psbcm", bufs=1, space="PSUM") as ps_bcm, \
             tc.tile_pool(name="pstp2", bufs=2, space="PSUM") as ps_tp2:

            x2 = p_x2.tile([P, 8, NLOC], F32, tag="x2")
            # preload x2 with full-precision x (xT_sb is fp32r-rounded; gpsimd
            # cast DMA copies the raw fp32 bits), then add c_proj output in place
            nc.gpsimd.dma_start(out=x2[:], in_=xT.rearrange("(a p) n -> p a n", p=P))
            for half in range(2):
                cw_sb = p_wmlp.tile([P, 8, 512], F32R, tag="w16")
                nc.sync.dma_start(out=cw_sb[:], in_=cproj_w[:, :, half * 512:(half + 1) * 512])
                for ft in range(4):
                    gft = half * 4 + ft
                    op = ps_mm.tile([P, NLOC], F32, tag="mm")
                    for ds in range(8):
                        nc.tensor.matmul(op[:], _r(cw_sb[:, ds, ft * 128:(ft + 1) * 128]),
                                         _r(yT[:, ds, :]), start=(ds == 0), stop=(ds == 7))
                    nc.vector.tensor_add(out=x2[:, gft, :], in0=op[:], in1=x2[:, gft, :])
            # x and y pools no longer needed (LIFO on right side: y then x)
            g_y.close()
            g_x.close()

            # rms2 (exact fp32 sumsq: feeds the gate)
            ss2 = ps_row2.tile([1, NLOC], F32, tag="row2")
            for ds in range(8):
                xsq = p_sc.tile([P, NLOC], F32, tag="xsq32")
                nc.vector.tensor_mul(out=xsq[:], in0=x2[:, ds, :], in1=x2[:, ds, :])
                nc.tensor.matmul(ss2[:], ones32_sb[:], xsq[:],
                                 start=(ds == 0), stop=(ds == 7))
            nc.scalar.activation(out=rows_sb[:, 1, :], in_=ss2[:],
                                 func=AF.Sqrt, bias=eps1[:], scale=1.0 / D)
            nc.vector.reciprocal(out=rows_sb[:, 0, :], in_=rows_sb[:, 1, :])
            h2 = p_x2.tile([P, 8, NLOC], F32R, tag="h2")
            s2b = brow(ps_bcm, p_sc, rows_sb[:, 0, :])
            for ds in range(8):
                nc.vector.tensor_tensor(out=h2[:, ds, :], in0=x2[:, ds, :],
                                        in1=s2b[:], op=AL.mult)

            # gate: exact fp32 logits -> softmax -> top2 combine weights
            gw_sb = p_const.tile([P, 8, E], F32, tag="gw")
            nc.sync.dma_start(out=gw_sb[:], in_=gate_w[:])
            gp = ps_mm.tile([P, NLOC], F32, tag="mm")
            for ds in range(8):
                nc.tensor.matmul(gp[:E], gw_sb[:, ds, :], x2[:, ds, :],
                                 start=(ds == 0), stop=(ds == 7))
            g_sb = p_sc.tile([E, NLOC], F32, tag="gsb")
            nc.vector.tensor_copy(out=g_sb[:], in_=gp[:E])
            cwT = p_const.tile([E, NLOC], F32, tag="cwT")
            for q4 in range(4):
                tp = ps_tp2.tile([P, P], F32, tag="tp2")
                nc.tensor.transpose(tp[:, 0:E], g_sb[:, q4 * 128:(q4 + 1) * 128],
                                    identf[0:E, 0:E])
                gt = p_sc.tile([P, E], F32, tag="gt")
                nc.vector.tensor_copy(out=gt[:], in_=tp[:, 0:E])
                s2tp = ps_tp2.tile([P, P], F32, tag="tp2")
                nc.tensor.transpose(s2tp[:, 0:1], rows_sb[:, 0, q4 * 128:(q4 + 1) * 128],
                                    identf[0:1, 0:1])
                s2c = p_sc.tile([P, 1], F32, tag="s2c")
                nc.vector.tensor_copy(out=s2c[:], in_=s2tp[:, 0:1])
                nc.vector.tensor_scalar_mul(out=gt[:], in0=gt[:], scalar1=s2c[:])
                mx = p_sc.tile([P, 4], F32, tag="mx")
                nc.vector.tensor_reduce(out=mx[:, 0:1], in_=gt[:], axis=mybir.AxisListType.X,
                                        op=AL.max)
                nc.vector.tensor_scalar_mul(out=mx[:, 1:2], in0=mx[:, 0:1], scalar1=-1.0)
                e8 = p_sc.tile([P, E], F32, tag="e8")
                nc.scalar.activation(out=e8[:], in_=gt[:], func=AF.Exp,
                                     bias=mx[:, 1:2], accum_out=mx[:, 2:3])
                nc.vector.reciprocal(out=mx[:, 3:4], in_=mx[:, 2:3])
                srt = p_sc.tile([P, E], F32, tag="srt")
                nc.vector.max(out=srt[:], in_=e8[:])
                cwq = p_sc.tile([P, E], F32, tag="cwq")
                nc.vector.tensor_scalar(out=cwq[:], in0=e8[:], scalar1=srt[:, 1:2],
                                        scalar2=None, op0=AL.is_ge)
                nc.vector.tensor_mul(out=cwq[:], in0=cwq[:], in1=e8[:])
                nc.vector.tensor_scalar_mul(out=cwq[:], in0=cwq[:], scalar1=mx[:, 3:4])
                tp2 = ps_tp2.tile([P, P], F32, tag="tp2")
                nc.tensor.transpose(tp2[0:E, 0:P], cwq[:], identf[:])
                nc.vector.tensor_copy(out=cwT[:, q4 * 128:(q4 + 1) * 128], in_=tp2[0:E, 0:P])

            # shared expert (weights streamed in 2MB halves, tag w16)
            macc = p_x2.tile([P, 8, NLOC], F32, tag="macc")
            hsh = p_x2.tile([P, 8, NLOC], F32R, tag="hsh")
            for half in range(2):
                w1_sb = p_wmlp.tile([P, 8, 512], F32R, tag="w16")
                nc.sync.dma_start(out=w1_sb[:], in_=shw1[:, :, half * 512:(half + 1) * 512])
                w3_sb = p_wmlp.tile([P, 8, 512], F32R, tag="w16")
                nc.sync.dma_start(out=w3_sb[:], in_=shw3[:, :, half * 512:(half + 1) * 512])
                for ft in range(4):
                    gft = half * 4 + ft
                    g1 = ps_mm.tile([P, NLOC], F32, tag="mm")
                    g3 = ps_mm.tile([P, NLOC], F32, tag="mm")
                    for ds in range(8):
                        nc.tensor.matmul(g1[:], _r(w1_sb[:, ds, ft * 128:(ft + 1) * 128]),
                                         _r(h2[:, ds, :]), start=(ds == 0), stop=(ds == 7))
                    for ds in range(8):
                        nc.tensor.matmul(g3[:], _r(w3_sb[:, ds, ft * 128:(ft + 1) * 128]),
                                         _r(h2[:, ds, :]), start=(ds == 0), stop=(ds == 7))
                    sl = p_sc.tile([P, NLOC], F32, tag="silu")
                    nc.scalar.activation(out=sl[:], in_=g1[:], func=AF.Silu)
                    nc.vector.tensor_mul(out=hsh[:, gft, :], in0=sl[:], in1=g3[:])
            for half in range(2):
                w2_sb = p_wmlp.tile([P, 8, 512], F32R, tag="w16")
                nc.sync.dma_start(out=w2_sb[:], in_=shw2[:, :, half * 512:(half + 1) * 512])
                for ft in range(4):
                    gft = half * 4 + ft
                    op = ps_mm.tile([P, NLOC], F32, tag="mm")
                    for ds in range(8):
                        nc.tensor.matmul(op[:], _r(w2_sb[:, ds, ft * 128:(ft + 1) * 128]),
                                         _r(hsh[:, ds, :]), start=(ds == 0), stop=(ds == 7))
                    nc.vector.tensor_add(out=macc[:, gft, :], in0=op[:], in1=x2[:, gft, :])

            # routed experts (dense: cw zeros non-selected)
            for e in range(E):
                e1_sb = p_wmlp.tile([P, 8, INTER], F32R, tag="w16")
                nc.sync.dma_start(out=e1_sb[:], in_=ew1[e])
                e3_sb = p_wmlp.tile([P, 8, INTER], F32R, tag="w16")
                nc.sync.dma_start(out=e3_sb[:], in_=ew3[e])
                he = p_x2.tile([P, 4, NLOC], F32R, tag="he")
                cwp = ps_bcm.tile([P, NLOC], F32, tag="bc")
                nc.tensor.matmul(cwp[:], sel8_sb[:, e * P:(e + 1) * P], cwT[:],
                                 start=True, stop=True)
                cwb = p_sc.tile([P, NLOC], F32, tag="bcsb")
                nc.vector.tensor_copy(out=cwb[:], in_=cwp[:])
                for ft in range(4):
                    g1 = ps_mm.tile([P, NLOC], F32, tag="mm")
                    g3 = ps_mm.tile([P, NLOC], F32, tag="mm")
                    for ds in range(8):
                        nc.tensor.matmul(g1[:], _r(e1_sb[:, ds, ft * 128:(ft + 1) * 128]),
                                         _r(h2[:, ds, :]), start=(ds == 0), stop=(ds == 7))
                    for ds in range(8):
                        nc.tensor.matmul(g3[:], _r(e3_sb[:, ds, ft * 128:(ft + 1) * 128]),
                                         _r(h2[:, ds, :]), start=(ds == 0), stop=(ds == 7))
                    sl = p_sc.tile([P, NLOC], F32, tag="silu")
                    nc.scalar.activation(out=sl[:], in_=g1[:], func=AF.Silu)
                    nc.vector.tensor_mul(out=he[:, ft, :], in0=sl[:], in1=g3[:])
                    nc.vector.tensor_tensor(out=he[:, ft, :], in0=he[:, ft, :],
                                            in1=cwb[:], op=AL.mult)
                e2_sb = p_wmlp.tile([P, 4, D], F32R, tag="w16")
                nc.sync.dma_start(out=e2_sb[:], in_=ew2[e])
                for ft in range(8):
                    op = ps_mm.tile([P, NLOC], F32, tag="mm")
                    for ds in range(4):
                        nc.tensor.matmul(op[:], _r(e2_sb[:, ds, ft * 128:(ft + 1) * 128]),
                                         _r(he[:, ds, :]), start=(ds == 0), stop=(ds == 3))
                    nc.vector.tensor_add(out=macc[:, ft, :], in0=op[:], in1=macc[:, ft, :])

            nc.sync.dma_start(out=out_xT.rearrange("(a p) n -> p a n", p=P), in_=macc[:])

    nc.finalize()
    return nc


# ============================ host side ============================

_CACHE = {}


def _prep_shared(inputs):
    perm = _rope_perm()
    latent_w = inputs["latent_w"] * inputs["rmsn1_w"][:, None]
    latent_w = latent_w.copy()
    latent_w[:, QLR + KVLR:] = latent_w[:, QLR + KVLR:][:, perm]
    q_up = inputs["q_up_w"] * inputs["q_norm_w"][:, None]
    q_up = q_up.copy()
    for h in range(H):
        c0 = h * QKD + NOPE
        q_up[:, c0:c0 + ROPE] = q_up[:, c0:c0 + ROPE][:, perm]
    kv_up = inputs["kv_up_w"] * inputs["kv_norm_w"][:, None]
    wv = np.concatenate([kv_up[:, h * (NOPE + VD) + NOPE:(h + 1) * (NOPE + VD)]
                         for h in range(H)], axis=1)  # [KVLR, H*VD]
    r2 = inputs["rmsn2_w"][:, None]
    f = np.float32
    shared = {
        "ident": np.eye(P, dtype=f),
        "ones32": np.ones((P, 1), dtype=f),
        "onesbf": np.ones((P, 1), dtype=ml_dtypes.bfloat16),
        "lat_w": _tile_w(latent_w.astype(f)),
        "q_up": _tile_w(q_up.astype(f)),
        "kv_up": _tile_w(kv_up.astype(f)),
        "wv_w": _tile_w(wv.astype(f)),
        "cproj_w": _tile_w(inputs["c_proj_w"].astype(f)),
        "gate_w": _tile_w((inputs["gate_w"] * r2).astype(f)),
        "shw1": _tile_w((inputs["sh_w1"] * r2).astype(f)),
        "shw3": _tile_w((inputs["sh_w3"] * r2).astype(f)),
        "shw2": _tile_w(inputs["sh_w2"].astype(f)),
        "ew1": np.stack([_tile_w((inputs["e_w1"][e] * r2).astype(f)) for e in range(E)]),
        "ew3": np.stack([_tile_w((inputs["e_w3"][e] * r2).astype(f)) for e in range(E)]),
        "ew2": np.stack([_tile_w(inputs["e_w2"][e].astype(f)) for e in range(E)]),
        "sel8": np.repeat(np.eye(E, dtype=f), P, axis=1).reshape(E, E * P),
    }
    return shared


def _prep_core(inputs, c):
    f = np.float32
    pos = _core_positions(c)
    b = c // 4
    gidx = b * T + pos
    xflat = np.asarray(inputs["x"], dtype=f).reshape(N, D)
    xT_c = np.ascontiguousarray(xflat[gidx].T)
    cosT = np.ascontiguousarray(np.asarray(inputs["freqs_cos"], f)[pos].T)
    sinT = np.ascontiguousarray(np.asarray(inputs["freqs_sin"], f)[pos].T)
    k_abs = (np.arange(8)[:, None] * 128 + np.arange(P)[None, :])  # [8,128]
    m1 = np.where(k_abs[:, :, None] <= pos[None, None, :], 0.0, NEG)
    k_abs2 = ((np.arange(8, 16))[:, None] * 128 + np.arange(P)[None, :])
    m2 = np.where(k_abs2[:, :, None] <= pos[None, None, 256:], 0.0, NEG)
    return {
        "xT": xT_c, "cosT": cosT, "sinT": sinT,
        "mask1": m1.astype(ml_dtypes.bfloat16),
        "mask2": m2.astype(ml_dtypes.bfloat16),
    }, gidx


def run(inputs, trace=False, **kw):
    if "nc" not in _CACHE:
        _CACHE["nc"] = build()
    nc = _CACHE["nc"]
    shared = _prep_shared({k: np.asarray(v) for k, v in inputs.items()
                           if k not in ("x", "freqs_cos", "freqs_sin")}
                          | {k: np.asarray(inputs[k]) for k in ("x", "freqs_cos", "freqs_sin")})
    in_maps = []
    gidxs = []
    for c in range(NCORES):
        m, gidx = _prep_core(inputs, c)
        m.update(shared)
        in_maps.append(m)
        gidxs.append(gidx)
    res = run_bass_kernel_spmd(nc, in_maps, core_ids=list(range(NCORES)),
                               trace=trace, **kw)
    full = np.empty((N, D), dtype=np.float32)
    for c in range(NCORES):
        full[gidxs[c]] = res.results[c]["out_xT"].T
    return full.reshape(B, T, D), res


def kernel(**inputs):
    out, _ = run(inputs)
    return out

